# revision 1
# baseline (speedup 1.0000x reference)
"""Trainium2 Bass kernel for edge-conv GNN message passing.

h = segment_sum(x[src] * (edge_basis @ W.T + b), dst, N)

Strategy (8 NeuronCores, SPMD single program):
  - Host: stable-sort edges by dst, split into 8 contiguous dst ranges of
    12500 nodes (core c owns dst in [12500c, 12500(c+1))). Within a core the
    edge stream is cut into fixed 768-edge segments; each segment's dst span
    is < 64 nodes (guaranteed statistically at ~8 sigma; asserted), so the
    segment aggregates into a [64, 64] node-window accumulator with a
    host-known base row.
  - Device, per 128-edge chunk: MM1 filt = ebT_chunk.T @ WT (PSUM), one
    dma_gather per 1536 edges fetches x pair-rows (512B) with signed int16
    indices centered at row 25000 of the [50000, 128] pair-packed x, DVE
    parity-select + multiply build mx = [m | xg], MM2 aggregates
    hseg += onehot.T @ mx into PSUM [64, 128] = [hm | hx].
  - Segment flush: PSUM -> SBUF -> DRAM slab [64, 128]. Host combines:
    h[base:base+64] += hm + hx * b  (bias folded via the hx plane).

Matmuls run in float32r (full-rate PE mode, ~1.5e-4 rel err) by default;
set FP32R = False for exact fp32 (4x slower PE).
"""

import numpy as np

# ---------------- problem constants (hardcoded per spec) ----------------
N_NODES = 100000
N_EDGES = 1600000
D_IN = 64
D_RADIAL = 128
N_CORES = 8
NODES_PER_CORE = N_NODES // N_CORES  # 12500

CHUNK = 128            # edges per matmul chunk (PE contraction dim)
SEG_CHUNKS = 6         # chunks per segment
SEG = CHUNK * SEG_CHUNKS            # 768 edges per segment
GROUP_SEGS = 2
GROUP = SEG * GROUP_SEGS            # 1536 edges per dma_gather call
N_GROUPS = 136
E_CAP = GROUP * N_GROUPS            # 208896 edge slots per core
N_SEGS = N_GROUPS * GROUP_SEGS      # 272 segments per core
N_CHUNKS = E_CAP // CHUNK           # 1632
WIN = 64               # nodes per segment accumulator window
PAIR_BASE = N_NODES // 4            # 25000: pair-row index bias
IDX_COLS = GROUP // 16              # 96 idx columns per group
FP32R = True

_CACHED = {}


def _build_nc(n_groups=N_GROUPS):
    import concourse.bass as bass
    import concourse.bacc as bacc
    import concourse.mybir as mybir
    from concourse.tile import TileContext

    f32 = mybir.dt.float32
    fcomp = mybir.dt.float32r if FP32R else f32

    e_cap = n_groups * GROUP
    n_segs = n_groups * GROUP_SEGS
    n_chunks = e_cap // CHUNK

    nc = bacc.Bacc(None, target_bir_lowering=False, debug=False)

    x_pair = nc.dram_tensor("x_pair", [N_NODES // 2, 2 * D_IN], f32, kind="ExternalInput")
    ebT = nc.dram_tensor("ebT", [D_RADIAL, e_cap], fcomp, kind="ExternalInput")
    WT = nc.dram_tensor("WT", [D_RADIAL, D_IN], fcomp, kind="ExternalInput")
    idxT = nc.dram_tensor("idxT", [128, IDX_COLS * n_groups], mybir.dt.int16, kind="ExternalInput")
    ldstT = nc.dram_tensor("ldstT", [128, n_chunks], fcomp, kind="ExternalInput")
    parT = nc.dram_tensor("parT", [128, n_chunks], mybir.dt.uint8, kind="ExternalInput")
    iota = nc.dram_tensor("iota", [128, SEG_CHUNKS, WIN], fcomp, kind="ExternalInput")
    slabs = nc.dram_tensor("slabs", [n_segs, WIN, 2 * D_IN], f32, kind="ExternalOutput")

    with TileContext(nc) as tc:
        with (
            tc.tile_pool(name="const", bufs=1) as cpool,
            tc.tile_pool(name="eb", bufs=2) as ebpool,
            tc.tile_pool(name="xg", bufs=2) as xgpool,
            tc.tile_pool(name="mx", bufs=3) as mxpool,
            tc.tile_pool(name="xsel", bufs=3) as xselpool,
            tc.tile_pool(name="oh", bufs=3) as ohpool,
            tc.tile_pool(name="stage", bufs=3) as stpool,
            tc.tile_pool(name="fps", bufs=2, space="PSUM") as fpool,
            tc.tile_pool(name="hps", bufs=2, space="PSUM") as hpool,
        ):
            WT_t = cpool.tile([D_RADIAL, D_IN], fcomp)
            nc.sync.dma_start(out=WT_t[:], in_=WT[:])
            iota_t = cpool.tile([128, SEG_CHUNKS, WIN], fcomp)
            nc.sync.dma_start(out=iota_t[:], in_=iota[:])
            idx_t = cpool.tile([128, IDX_COLS * n_groups], mybir.dt.int16)
            nc.sync.dma_start(out=idx_t[:], in_=idxT[:])
            ldst_t = cpool.tile([128, n_chunks], fcomp)
            nc.sync.dma_start(out=ldst_t[:], in_=ldstT[:])
            par_t = cpool.tile([128, n_chunks], mybir.dt.uint8)
            nc.sync.dma_start(out=par_t[:], in_=parT[:])

            for g in range(n_groups):
                ebtile = ebpool.tile([D_RADIAL, GROUP], fcomp)
                nc.sync.dma_start(out=ebtile[:], in_=ebT[:, g * GROUP:(g + 1) * GROUP])
                xg = xgpool.tile([128, GROUP // 128, 2 * D_IN], f32)
                nc.gpsimd.dma_gather(
                    out_ap=xg[:],
                    in_ap=x_pair[PAIR_BASE:],
                    idxs_ap=idx_t[:, g * IDX_COLS:(g + 1) * IDX_COLS],
                    num_idxs=GROUP,
                    num_idxs_reg=GROUP,
                    elem_size=2 * D_IN,
                    single_packet=False,
                )
                for half in range(GROUP_SEGS):
                    s = g * GROUP_SEGS + half
                    c0 = s * SEG_CHUNKS  # global chunk index of segment start

                    filt_ps = fpool.tile([128, SEG_CHUNKS, D_IN], mybir.dt.float32)
                    for j in range(SEG_CHUNKS):
                        nc.tensor.matmul(
                            filt_ps[:, j],
                            ebtile[:, (half * SEG_CHUNKS + j) * CHUNK:(half * SEG_CHUNKS + j + 1) * CHUNK],
                            WT_t[:],
                            start=True,
                            stop=True,
                        )

                    # parity-select the gathered pair halves into xsel (f32:
                    # CopyPredicated's ISA rejects f32r data)
                    xsel = xselpool.tile([128, SEG_CHUNKS, D_IN], f32)
                    nc.scalar.copy(
                        out=xsel[:],
                        in_=xg[:, half * SEG_CHUNKS:(half + 1) * SEG_CHUNKS, :D_IN],
                    )
                    nc.vector.copy_predicated(
                        out=xsel[:],
                        mask=par_t[:, c0:c0 + SEG_CHUNKS].to_broadcast(
                            [128, SEG_CHUNKS, D_IN]
                        ),
                        data=xg[:, half * SEG_CHUNKS:(half + 1) * SEG_CHUNKS, D_IN:],
                    )
                    mx = mxpool.tile([128, SEG_CHUNKS, 2 * D_IN], fcomp)
                    # xg plane (rounded to fcomp on ACT)
                    nc.scalar.copy(out=mx[:, :, D_IN:], in_=xsel[:])
                    # m = filt * xg  -> mx[:, :, :64]
                    nc.vector.tensor_tensor(
                        out=mx[:, :, :D_IN],
                        in0=filt_ps[:],
                        in1=xsel[:],
                        op=mybir.AluOpType.mult,
                    )
                    oh = ohpool.tile([128, SEG_CHUNKS, WIN], fcomp)
                    nc.vector.tensor_tensor(
                        out=oh[:],
                        in0=iota_t[:],
                        in1=ldst_t[:, c0:c0 + SEG_CHUNKS].to_broadcast(
                            [128, SEG_CHUNKS, WIN]
                        ),
                        op=mybir.AluOpType.is_equal,
                    )
                    hseg = hpool.tile([WIN, 2 * D_IN], mybir.dt.float32)
                    for j in range(SEG_CHUNKS):
                        nc.tensor.matmul(
                            hseg[:],
                            oh[:, j],
                            mx[:, j, :],
                            start=(j == 0),
                            stop=(j == SEG_CHUNKS - 1),
                        )
                    stage = stpool.tile([WIN, 2 * D_IN], mybir.dt.float32)
                    nc.scalar.copy(out=stage[:], in_=hseg[:])
                    nc.sync.dma_start(out=slabs[s], in_=stage[:])

    nc.finalize()
    return nc


def _host_preprocess(x, edge_basis, src, dst, W):
    """Shard + sort + pack per-core device inputs. Returns (in_maps, bases)."""
    src = np.ascontiguousarray(src).astype(np.int64)
    dst = np.ascontiguousarray(dst).astype(np.int64)
    x = np.ascontiguousarray(x, dtype=np.float32)
    edge_basis = np.ascontiguousarray(edge_basis, dtype=np.float32)
    W = np.ascontiguousarray(W, dtype=np.float32)

    order = np.argsort(dst, kind="stable")
    dst_s = dst[order]
    src_s = src[order]

    core_lo = np.searchsorted(dst_s, np.arange(N_CORES) * NODES_PER_CORE)
    core_hi = np.searchsorted(dst_s, (np.arange(N_CORES) + 1) * NODES_PER_CORE)

    x_pair = x.reshape(N_NODES // 2, 2 * D_IN)
    WT_h = W.T.copy()  # [128, 64]
    iota_h = np.tile(
        np.arange(WIN, dtype=np.float32), (128, SEG_CHUNKS, 1)
    )

    in_maps = []
    bases_all = []
    for c in range(N_CORES):
        lo, hi = core_lo[c], core_hi[c]
        n_real = hi - lo
        ldst_c = dst_s[lo:hi] - c * NODES_PER_CORE
        src_c = src_s[lo:hi]
        eb_idx = order[lo:hi]

        # ---- place edges into segment slots ----
        # each segment has SEG slots; the last slot of each GROUP is reserved
        # for padding (keeps the dma_gather trailing-negative skip disarmed).
        # padding slots use src = 2*PAIR_BASE so their gather index is 0
        # (>= 0: keeps dma_gather's trailing-negative skip disarmed)
        slot_src = np.full(E_CAP, 2 * PAIR_BASE, dtype=np.int64)
        slot_ldst_rel = np.full(E_CAP, -1.0, dtype=np.float32)
        slot_eb_row = np.full(E_CAP, -1, dtype=np.int64)    # -1 -> zero row

        # reserved slots: last slot of each group
        reserved = (np.arange(N_GROUPS) + 1) * GROUP - 1
        ok_mask = np.ones(E_CAP, dtype=bool)
        ok_mask[reserved] = False

        # greedy segmentation: fill segments with up to SEG usable slots,
        # breaking a segment early if its dst span would reach WIN nodes.
        usable = np.flatnonzero(ok_mask)  # usable slot ids in order
        # segment id of each usable slot
        seg_of_slot = usable // SEG

        bases = np.zeros(N_SEGS, dtype=np.int64)
        # assign edges to consecutive usable slots, but force a segment break
        # when dst span reaches WIN. With density ~16 edges/node, a 762-edge
        # segment spans ~48 nodes; span >= WIN is ~8 sigma out.
        pos = 0  # index into usable[]
        e = 0    # index into edge stream
        seg_start_node = -1
        cur_seg = 0
        bases[:] = 0
        while e < n_real:
            if pos >= len(usable):
                raise RuntimeError("E_CAP exceeded during segmentation")
            slot = usable[pos]
            seg = seg_of_slot[pos]
            node = ldst_c[e]
            if seg != cur_seg:
                cur_seg = seg
                seg_start_node = -1
            if seg_start_node < 0:
                seg_start_node = node
                bases[seg] = node
            if node - seg_start_node >= WIN:
                # advance pos to the start of the next segment
                nxt = np.searchsorted(seg_of_slot, seg + 1, side="left")
                if nxt <= pos:
                    raise RuntimeError("segmentation stuck")
                pos = nxt
                continue
            slot_src[slot] = src_c[e]
            slot_ldst_rel[slot] = node - seg_start_node
            slot_eb_row[slot] = eb_idx[e]
            pos += 1
            e += 1

        # ---- build packed arrays ----
        eb_pad = np.zeros((E_CAP, D_RADIAL), dtype=np.float32)
        filled = slot_eb_row >= 0
        eb_pad[filled] = edge_basis[slot_eb_row[filled]]
        ebT_c = np.ascontiguousarray(eb_pad.T)  # [128, E_CAP]

        # gather indices: pair row, centered
        pair_idx = (slot_src >> 1) - PAIR_BASE
        assert pair_idx.min() >= -32768 and pair_idx.max() < 32768
        parity = (slot_src & 1).astype(np.float32)

        # idx16 per group: idx16[q, f] = idx of edge slot (p, c) with
        # p = q + 16*(f%8), c = f//8   (slot-in-group = c*128 + p)
        pi = pair_idx.reshape(N_GROUPS, GROUP // 128, 128)  # [g, c, p]
        # build [g, q, f]: f = (p//16) + 8*c  -> for q in 0..15, f in 0..IDX_COLS-1
        idx16 = np.zeros((N_GROUPS, 16, IDX_COLS), dtype=np.int16)
        f_idx = np.arange(IDX_COLS)
        c_of_f = f_idx // 8
        phi_of_f = f_idx % 8  # p//16
        for q in range(16):
            p_of_f = q + 16 * phi_of_f
            idx16[:, q, :] = pi[:, c_of_f, p_of_f].astype(np.int16)
        idxT_c = np.tile(idx16, (1, 8, 1)).reshape(N_GROUPS, 128, IDX_COLS)
        idxT_c = np.ascontiguousarray(
            idxT_c.transpose(1, 0, 2).reshape(128, N_GROUPS * IDX_COLS)
        )

        # ldstT / parT: [128, N_CHUNKS] with column = global chunk, row = p
        ldstT_c = np.ascontiguousarray(
            slot_ldst_rel.reshape(N_CHUNKS, 128).T
        )
        parT_c = np.ascontiguousarray(parity.reshape(N_CHUNKS, 128).T.astype(np.uint8))

        in_maps.append(
            {
                "x_pair": x_pair,
                "ebT": ebT_c,
                "WT": WT_h,
                "idxT": idxT_c,
                "ldstT": ldstT_c,
                "parT": parT_c,
                "iota": iota_h,
            }
        )
        bases_all.append(bases)
    return in_maps, bases_all


def kernel(x, edge_basis, src, dst, W, b):
    from concourse.bass_utils import run_bass_kernel_spmd

    b = np.ascontiguousarray(b, dtype=np.float32)
    in_maps, bases_all = _host_preprocess(x, edge_basis, src, dst, W)

    if "nc" not in _CACHED:
        _CACHED["nc"] = _build_nc()
    nc = _CACHED["nc"]

    res = run_bass_kernel_spmd(nc, in_maps, core_ids=list(range(N_CORES)))

    h = np.zeros((N_NODES, D_IN), dtype=np.float32)
    for c in range(N_CORES):
        slabs = res.results[c]["slabs"]  # [N_SEGS, WIN, 128]
        bases = bases_all[c]
        h_pad = np.zeros((NODES_PER_CORE + WIN, D_IN), dtype=np.float32)
        for s in range(N_SEGS):
            sl = slabs[s]
            h_pad[bases[s]:bases[s] + WIN] += sl[:, :D_IN] + sl[:, D_IN:] * b
        h[c * NODES_PER_CORE:(c + 1) * NODES_PER_CORE] = h_pad[:NODES_PER_CORE]
    return h



# revision 3
# speedup vs baseline: 5.6667x; 5.6667x over previous
"""Trainium2 Bass kernel for edge-conv GNN message passing (V2, bf16).

h = segment_sum(x[src] * (edge_basis @ W.T + b), dst, N)

Strategy (8 NeuronCores, SPMD single program):
  - Host: stable-sort edges by dst, shard dst ranges of 12500 nodes per core.
    Fixed 768-edge segments; each segment's dst span < 64 nodes (verified;
    greedy fallback on violation). Host pre-gathers x[src] into edge-slot
    order (xgP, bf16) and pre-permutes edge_basis into slot order (ebT,
    bf16) so the device streams everything sequentially - no device gather.
  - Device per 128-edge chunk: MM1 filt = eb_chunk.T @ WT (PSUM f32),
    DVE m = filt * xg (bf16), gpsimd one-hot oh = (iota == ldst), MM2
    hseg += oh.T @ m into PSUM [64, 64]. ACT stages hseg to a per-group
    slab buffer (bf16), one slab DMA per group.
  - PE stream is software-pipelined with LAG segments between MM1 and MM2
    so MM2 never waits on the DVE multiply latency.
  - Bias: h += b * segment_sum(x[src]) is computed on host (exact f32 via
    np.add.reduceat over the dst-sorted gather), so the device only
    aggregates m.
"""

import numpy as np
import ml_dtypes

BF16 = ml_dtypes.bfloat16

# ---------------- problem constants (hardcoded per spec) ----------------
N_NODES = 100000
N_EDGES = 1600000
D_IN = 64
D_RADIAL = 128
N_CORES = 8
NODES_PER_CORE = N_NODES // N_CORES  # 12500

CHUNK = 128            # edges per matmul chunk (PE contraction dim)
SEG_CHUNKS = 6         # chunks per segment
SEG = CHUNK * SEG_CHUNKS            # 768 edges per segment
GROUP_SEGS = 16
GROUP = SEG * GROUP_SEGS            # 12288 edges per group
N_GROUPS = 17
E_CAP = GROUP * N_GROUPS            # 208896 edge slots per core
N_SEGS = N_GROUPS * GROUP_SEGS      # 272 segments per core
N_CHUNKS = E_CAP // CHUNK           # 1632
WIN = 64               # nodes per segment accumulator window
LAG = 2                # segments of PE software-pipelining

EB_BUFS = 3
XG_BUFS = 3
FILT_BUFS = 4
HSEG_BUFS = 3
M_BUFS = 4
OH_BUFS = 6
ST_BUFS = 3

_CACHED = {}


def _build_nc(n_groups=N_GROUPS):
    import concourse.bass as bass
    import concourse.bacc as bacc
    import concourse.mybir as mybir
    from concourse.tile import TileContext

    f32 = mybir.dt.float32
    bf16 = mybir.dt.bfloat16

    e_cap = n_groups * GROUP
    n_segs = n_groups * GROUP_SEGS
    n_chunks = e_cap // CHUNK

    nc = bacc.Bacc(None, target_bir_lowering=False, debug=False)

    ebT = nc.dram_tensor("ebT", [D_RADIAL, e_cap], bf16, kind="ExternalInput")
    xgP = nc.dram_tensor("xgP", [128, n_chunks * D_IN], bf16, kind="ExternalInput")
    WT = nc.dram_tensor("WT", [D_RADIAL, D_IN], bf16, kind="ExternalInput")
    ldstT = nc.dram_tensor("ldstT", [128, n_chunks], bf16, kind="ExternalInput")
    iota = nc.dram_tensor("iota", [128, SEG_CHUNKS, WIN], bf16, kind="ExternalInput")
    slabs = nc.dram_tensor(
        "slabs", [n_groups, WIN, GROUP_SEGS * D_IN], bf16, kind="ExternalOutput"
    )

    with TileContext(nc) as tc:
        with (
            tc.tile_pool(name="const", bufs=1) as cpool,
            tc.tile_pool(name="eb", bufs=EB_BUFS) as ebpool,
            tc.tile_pool(name="xg", bufs=XG_BUFS) as xgpool,
            tc.tile_pool(name="m", bufs=M_BUFS) as mpool,
            tc.tile_pool(name="oh", bufs=OH_BUFS) as ohpool,
            tc.tile_pool(name="stage", bufs=ST_BUFS) as stpool,
            tc.tile_pool(name="fps", bufs=FILT_BUFS, space="PSUM") as fpool,
            tc.tile_pool(name="hps", bufs=HSEG_BUFS, space="PSUM") as hpool,
        ):
            WT_t = cpool.tile([D_RADIAL, D_IN], bf16)
            nc.sync.dma_start(out=WT_t[:], in_=WT[:])
            iota_t = cpool.tile([128, SEG_CHUNKS, WIN], bf16)
            nc.sync.dma_start(out=iota_t[:], in_=iota[:])
            ldst_t = cpool.tile([128, n_chunks], bf16)
            nc.sync.dma_start(out=ldst_t[:], in_=ldstT[:])

            ebtiles = {}
            xgtiles = {}
            stages = {}
            filts = {}
            ms = {}
            ohs = {}

            def front(s):
                g, s_l = divmod(s, GROUP_SEGS)
                if s_l == 0:
                    ebtile = ebpool.tile([128, GROUP], bf16, name="ebtile")
                    nc.sync.dma_start(
                        out=ebtile[:], in_=ebT[:, g * GROUP:(g + 1) * GROUP]
                    )
                    ebtiles[g] = ebtile
                    xgt = xgpool.tile([128, GROUP_SEGS, SEG_CHUNKS, D_IN], bf16, name="xgt")
                    nc.sync.dma_start(
                        out=xgt[:],
                        in_=xgP[:, g * GROUP_SEGS * SEG_CHUNKS * D_IN:
                                (g + 1) * GROUP_SEGS * SEG_CHUNKS * D_IN],
                    )
                    xgtiles[g] = xgt
                    stages[g] = stpool.tile([WIN, GROUP_SEGS, D_IN], bf16, name="stage")
                c0 = s * SEG_CHUNKS
                # one-hot for this segment (no data deps; gpsimd runs ahead)
                oh = ohpool.tile([128, SEG_CHUNKS, WIN], bf16, name="oh")
                nc.vector.tensor_tensor(
                    out=oh[:],
                    in0=iota_t[:],
                    in1=ldst_t[:, c0:c0 + SEG_CHUNKS].to_broadcast(
                        [128, SEG_CHUNKS, WIN]
                    ),
                    op=mybir.AluOpType.is_equal,
                )
                ohs[s] = oh
                filt_ps = fpool.tile([128, SEG_CHUNKS, D_IN], f32, name="filt_ps")
                for j in range(SEG_CHUNKS):
                    nc.tensor.matmul(
                        filt_ps[:, j],
                        ebtiles[g][:, (s_l * SEG_CHUNKS + j) * CHUNK:
                                   (s_l * SEG_CHUNKS + j + 1) * CHUNK],
                        WT_t[:],
                        start=True,
                        stop=True,
                    )
                filts[s] = filt_ps
                m = mpool.tile([128, SEG_CHUNKS, D_IN], bf16, name="m")
                nc.vector.tensor_tensor(
                    out=m[:],
                    in0=filt_ps[:],
                    in1=xgtiles[g][:, s_l],
                    op=mybir.AluOpType.mult,
                )
                ms[s] = m

            def back(s):
                g, s_l = divmod(s, GROUP_SEGS)
                hseg = hpool.tile([WIN, D_IN], f32, name="hseg")
                for j in range(SEG_CHUNKS):
                    nc.tensor.matmul(
                        hseg[:],
                        ohs[s][:, j],
                        ms[s][:, j],
                        start=(j == 0),
                        stop=(j == SEG_CHUNKS - 1),
                    )
                del ohs[s], ms[s], filts[s]
                nc.scalar.copy(out=stages[g][:, s_l], in_=hseg[:])
                if s_l == GROUP_SEGS - 1:
                    nc.sync.dma_start(out=slabs[g], in_=stages[g][:])
                    del ebtiles[g], xgtiles[g], stages[g]

            for s in range(n_segs + LAG):
                if s < n_segs:
                    front(s)
                if s >= LAG:
                    back(s - LAG)

    nc.finalize()
    return nc


def _segment_bases(ldst_c):
    """Per-768-edge-segment window bases; greedy fallback if a span >= WIN."""
    n_real = len(ldst_c)
    n_full = (n_real + SEG - 1) // SEG
    bases = np.zeros(N_SEGS, dtype=np.int64)
    if n_real == 0:
        return bases, np.full(E_CAP, -1.0, dtype=np.float32), np.arange(0)
    starts = np.arange(n_full) * SEG
    ends = np.minimum(starts + SEG, n_real) - 1
    b = ldst_c[starts]
    spans = ldst_c[ends] - b
    if spans.max() < WIN:
        bases[:n_full] = b
        rel = np.full(E_CAP, -1.0, dtype=np.float32)
        seg_of = np.arange(n_real) // SEG
        rel[:n_real] = ldst_c - b[seg_of]
        return bases, rel, np.arange(n_real)
    # rare fallback: greedy with early segment breaks
    rel = np.full(E_CAP, -1.0, dtype=np.float32)
    slot_of_edge = np.zeros(n_real, dtype=np.int64)
    pos = 0
    e = 0
    seg_start_node = -1
    cur_seg = 0
    while e < n_real:
        if pos >= E_CAP:
            raise RuntimeError("E_CAP exceeded during segmentation")
        seg = pos // SEG
        node = ldst_c[e]
        if seg != cur_seg:
            cur_seg = seg
            seg_start_node = -1
        if seg_start_node < 0:
            seg_start_node = node
            bases[seg] = node
        if node - seg_start_node >= WIN:
            pos = (seg + 1) * SEG
            continue
        rel[pos] = node - seg_start_node
        slot_of_edge[e] = pos
        pos += 1
        e += 1
    return bases, rel, slot_of_edge


def _host_preprocess(x, edge_basis, src, dst, W):
    """Shard + sort + pack per-core device inputs. Returns (in_maps, sides)."""
    src = np.ascontiguousarray(src).astype(np.int64)
    dst = np.ascontiguousarray(dst).astype(np.int64)
    x = np.ascontiguousarray(x, dtype=np.float32)
    W = np.ascontiguousarray(W, dtype=np.float32)

    order = np.argsort(dst, kind="stable")
    dst_s = dst[order]
    src_s = src[order]

    core_lo = np.searchsorted(dst_s, np.arange(N_CORES) * NODES_PER_CORE)
    core_hi = np.searchsorted(dst_s, (np.arange(N_CORES) + 1) * NODES_PER_CORE)

    x_bf = x.astype(BF16)
    eb_bf = np.asarray(edge_basis).astype(BF16)
    WT_h = np.ascontiguousarray(W.T.astype(BF16))  # [128, 64]
    iota_h = np.tile(
        np.arange(WIN, dtype=np.float32).astype(BF16), (128, SEG_CHUNKS, 1)
    )

    in_maps = []
    sides = []
    for c in range(N_CORES):
        lo, hi = core_lo[c], core_hi[c]
        n_real = hi - lo
        ldst_c = dst_s[lo:hi] - c * NODES_PER_CORE
        src_c = src_s[lo:hi]
        eb_idx = order[lo:hi]

        bases, rel, slot_of_edge = _segment_bases(ldst_c)

        # slot -> edge id (or -1)
        slot_edge = np.full(E_CAP, -1, dtype=np.int64)
        slot_edge[slot_of_edge] = np.arange(n_real)

        filled = slot_edge >= 0
        # ---- ebT: [128, E_CAP] bf16, zero on padding ----
        eb_pad = np.zeros((E_CAP, D_RADIAL), dtype=BF16)
        eb_pad[filled] = eb_bf[eb_idx[slot_edge[filled]]]
        ebT_c = np.ascontiguousarray(eb_pad.T)

        # ---- xgP: [128, n_chunks*64] bf16 (partition = edge-in-chunk) ----
        slot_src = np.zeros(E_CAP, dtype=np.int64)
        slot_src[filled] = src_c[slot_edge[filled]]
        sp = slot_src.reshape(N_CHUNKS, 128).T  # [128, n_chunks]
        xgP_c = np.ascontiguousarray(
            x_bf[sp].reshape(128, N_CHUNKS * D_IN)
        )

        # ---- ldstT: [128, n_chunks] bf16 ----
        ldstT_c = np.ascontiguousarray(rel.reshape(N_CHUNKS, 128).T).astype(BF16)

        in_maps.append(
            {
                "ebT": ebT_c,
                "xgP": xgP_c,
                "WT": WT_h,
                "ldstT": ldstT_c,
                "iota": iota_h,
            }
        )

        # host-side bias term: hb[n] = sum_{e: dst=n} x[src_e] (f32 exact)
        xb = np.zeros((NODES_PER_CORE, D_IN), dtype=np.float32)
        if n_real > 0:
            runs = np.flatnonzero(np.diff(ldst_c)) + 1
            boundaries = np.concatenate(([0], runs))
            sums = np.add.reduceat(x[src_c], boundaries, axis=0)
            xb[ldst_c[boundaries]] = sums
        sides.append((bases, xb))
    return in_maps, sides


def kernel(x, edge_basis, src, dst, W, b):
    from concourse.bass_utils import run_bass_kernel_spmd

    b = np.ascontiguousarray(b, dtype=np.float32)
    in_maps, sides = _host_preprocess(x, edge_basis, src, dst, W)

    if "nc" not in _CACHED:
        _CACHED["nc"] = _build_nc()
    nc = _CACHED["nc"]

    res = run_bass_kernel_spmd(nc, in_maps, core_ids=list(range(N_CORES)))

    h = np.zeros((N_NODES, D_IN), dtype=np.float32)
    for c in range(N_CORES):
        slabs = np.asarray(res.results[c]["slabs"], dtype=np.float32)
        # [n_groups, WIN, GROUP_SEGS*64] -> [n_segs, WIN, 64]
        slabs = slabs.reshape(N_GROUPS, WIN, GROUP_SEGS, D_IN)
        slabs = slabs.transpose(0, 2, 1, 3).reshape(N_SEGS, WIN, D_IN)
        bases, xb = sides[c]
        h_pad = np.zeros((NODES_PER_CORE + WIN, D_IN), dtype=np.float32)
        for s in range(N_SEGS):
            h_pad[bases[s]:bases[s] + WIN] += slabs[s]
        hc = h_pad[:NODES_PER_CORE]
        hc += xb * b
        h[c * NODES_PER_CORE:(c + 1) * NODES_PER_CORE] = hc
    return h


# revision 4
# speedup vs baseline: 6.5756x; 1.1604x over previous
"""Trainium2 Bass kernel for edge-conv GNN message passing (V2.1, bf16).

h = segment_sum(x[src] * (edge_basis @ W.T + b), dst, N)

See kernel_v2 docstring for the core design. V2.1 changes:
  - dynamic n_groups sized to the actual max per-core edge count
  - GROUP_SEGS=8 (smaller groups: earlier start, finer DMA pipelining)
  - is_equal one-hot batched per 2 segments (halves DVE fixed overhead)
  - LAG=3 with deeper tile pools
"""

import numpy as np
import ml_dtypes

BF16 = ml_dtypes.bfloat16

# ---------------- problem constants (hardcoded per spec) ----------------
N_NODES = 100000
N_EDGES = 1600000
D_IN = 64
D_RADIAL = 128
N_CORES = 8
NODES_PER_CORE = N_NODES // N_CORES  # 12500

CHUNK = 128            # edges per matmul chunk (PE contraction dim)
SEG_CHUNKS = 6         # chunks per segment
SEG = CHUNK * SEG_CHUNKS            # 768 edges per segment
GROUP_SEGS = 8
GROUP = SEG * GROUP_SEGS            # 6144 edges per group
WIN = 64               # nodes per segment accumulator window
LAG = 3                # segments of PE software-pipelining

EB_BUFS = 3
XG_BUFS = 3
FILT_BUFS = 5
HSEG_BUFS = 3
M_BUFS = 6
OH_BUFS = 3            # each oh tile covers 2 segments
ST_BUFS = 3

_CACHED = {}


def _build_nc(n_groups):
    import concourse.bacc as bacc
    import concourse.mybir as mybir
    from concourse.tile import TileContext

    f32 = mybir.dt.float32
    bf16 = mybir.dt.bfloat16

    e_cap = n_groups * GROUP
    n_segs = n_groups * GROUP_SEGS
    n_chunks = e_cap // CHUNK

    nc = bacc.Bacc(None, target_bir_lowering=False, debug=False)

    ebT = nc.dram_tensor("ebT", [D_RADIAL, e_cap], bf16, kind="ExternalInput")
    xgP = nc.dram_tensor("xgP", [128, n_chunks * D_IN], bf16, kind="ExternalInput")
    WT = nc.dram_tensor("WT", [D_RADIAL, D_IN], bf16, kind="ExternalInput")
    ldstT = nc.dram_tensor("ldstT", [128, n_chunks], bf16, kind="ExternalInput")
    iota = nc.dram_tensor(
        "iota", [128, 2 * SEG_CHUNKS, WIN], bf16, kind="ExternalInput"
    )
    slabs = nc.dram_tensor(
        "slabs", [n_groups, WIN, GROUP_SEGS * D_IN], bf16, kind="ExternalOutput"
    )

    with TileContext(nc) as tc:
        with (
            tc.tile_pool(name="const", bufs=1) as cpool,
            tc.tile_pool(name="eb", bufs=EB_BUFS) as ebpool,
            tc.tile_pool(name="xg", bufs=XG_BUFS) as xgpool,
            tc.tile_pool(name="m", bufs=M_BUFS) as mpool,
            tc.tile_pool(name="oh", bufs=OH_BUFS) as ohpool,
            tc.tile_pool(name="stage", bufs=ST_BUFS) as stpool,
            tc.tile_pool(name="fps", bufs=FILT_BUFS, space="PSUM") as fpool,
            tc.tile_pool(name="hps", bufs=HSEG_BUFS, space="PSUM") as hpool,
        ):
            WT_t = cpool.tile([D_RADIAL, D_IN], bf16)
            nc.sync.dma_start(out=WT_t[:], in_=WT[:])
            iota_t = cpool.tile([128, 2 * SEG_CHUNKS, WIN], bf16)
            nc.sync.dma_start(out=iota_t[:], in_=iota[:])
            ldst_t = cpool.tile([128, n_chunks], bf16)
            nc.sync.dma_start(out=ldst_t[:], in_=ldstT[:])

            ebtiles = {}
            xgtiles = {}
            stages = {}
            ms = {}
            ohs = {}

            def front(s):
                g, s_l = divmod(s, GROUP_SEGS)
                if s_l == 0:
                    ebtile = ebpool.tile([128, GROUP], bf16, name="ebtile")
                    nc.sync.dma_start(
                        out=ebtile[:], in_=ebT[:, g * GROUP:(g + 1) * GROUP]
                    )
                    ebtiles[g] = ebtile
                    xgt = xgpool.tile(
                        [128, GROUP_SEGS, SEG_CHUNKS, D_IN], bf16, name="xgt"
                    )
                    nc.sync.dma_start(
                        out=xgt[:],
                        in_=xgP[:, g * GROUP_SEGS * SEG_CHUNKS * D_IN:
                                (g + 1) * GROUP_SEGS * SEG_CHUNKS * D_IN],
                    )
                    xgtiles[g] = xgt
                    stages[g] = stpool.tile(
                        [WIN, GROUP_SEGS, D_IN], bf16, name="stage"
                    )
                c0 = s * SEG_CHUNKS
                if s % 2 == 0:
                    # one-hot for this segment pair (no deps; DVE runs ahead)
                    oh = ohpool.tile([128, 2 * SEG_CHUNKS, WIN], bf16, name="oh")
                    nc.vector.tensor_tensor(
                        out=oh[:],
                        in0=iota_t[:],
                        in1=ldst_t[:, c0:c0 + 2 * SEG_CHUNKS].to_broadcast(
                            [128, 2 * SEG_CHUNKS, WIN]
                        ),
                        op=mybir.AluOpType.is_equal,
                    )
                    ohs[s] = (oh, 0)
                    ohs[s + 1] = (oh, SEG_CHUNKS)
                filt_ps = fpool.tile([128, SEG_CHUNKS, D_IN], f32, name="filt_ps")
                for j in range(SEG_CHUNKS):
                    nc.tensor.matmul(
                        filt_ps[:, j],
                        ebtiles[g][:, (s_l * SEG_CHUNKS + j) * CHUNK:
                                   (s_l * SEG_CHUNKS + j + 1) * CHUNK],
                        WT_t[:],
                        start=True,
                        stop=True,
                    )
                m = mpool.tile([128, SEG_CHUNKS, D_IN], bf16, name="m")
                nc.vector.tensor_tensor(
                    out=m[:],
                    in0=filt_ps[:],
                    in1=xgtiles[g][:, s_l],
                    op=mybir.AluOpType.mult,
                )
                ms[s] = m

            def back(s):
                g, s_l = divmod(s, GROUP_SEGS)
                oh, joff = ohs[s]
                hseg = hpool.tile([WIN, D_IN], f32, name="hseg")
                for j in range(SEG_CHUNKS):
                    nc.tensor.matmul(
                        hseg[:],
                        oh[:, joff + j],
                        ms[s][:, j],
                        start=(j == 0),
                        stop=(j == SEG_CHUNKS - 1),
                    )
                del ohs[s], ms[s]
                nc.scalar.copy(out=stages[g][:, s_l], in_=hseg[:])
                if s_l == GROUP_SEGS - 1:
                    nc.sync.dma_start(out=slabs[g], in_=stages[g][:])
                    del ebtiles[g], xgtiles[g], stages[g]

            for s in range(n_segs + LAG):
                if s < n_segs:
                    front(s)
                if s >= LAG:
                    back(s - LAG)

    nc.finalize()
    return nc


def _segment_bases(ldst_c, n_segs, e_cap):
    """Per-768-edge-segment window bases; greedy fallback if a span >= WIN."""
    n_real = len(ldst_c)
    n_full = (n_real + SEG - 1) // SEG
    bases = np.zeros(n_segs, dtype=np.int64)
    if n_real == 0:
        return bases, np.full(e_cap, -1.0, dtype=np.float32), np.arange(0)
    starts = np.arange(n_full) * SEG
    ends = np.minimum(starts + SEG, n_real) - 1
    b = ldst_c[starts]
    spans = ldst_c[ends] - b
    if spans.max() < WIN:
        bases[:n_full] = b
        rel = np.full(e_cap, -1.0, dtype=np.float32)
        seg_of = np.arange(n_real) // SEG
        rel[:n_real] = ldst_c - b[seg_of]
        return bases, rel, np.arange(n_real)
    # rare fallback: greedy with early segment breaks
    rel = np.full(e_cap, -1.0, dtype=np.float32)
    slot_of_edge = np.zeros(n_real, dtype=np.int64)
    pos = 0
    e = 0
    seg_start_node = -1
    cur_seg = 0
    while e < n_real:
        if pos >= e_cap:
            raise RuntimeError("e_cap exceeded during segmentation")
        seg = pos // SEG
        node = ldst_c[e]
        if seg != cur_seg:
            cur_seg = seg
            seg_start_node = -1
        if seg_start_node < 0:
            seg_start_node = node
            bases[seg] = node
        if node - seg_start_node >= WIN:
            pos = (seg + 1) * SEG
            continue
        rel[pos] = node - seg_start_node
        slot_of_edge[e] = pos
        pos += 1
        e += 1
    return bases, rel, slot_of_edge


def _host_preprocess(x, edge_basis, src, dst, W):
    """Shard + sort + pack per-core device inputs.

    Returns (in_maps, sides, n_groups)."""
    src = np.ascontiguousarray(src).astype(np.int64)
    dst = np.ascontiguousarray(dst).astype(np.int64)
    x = np.ascontiguousarray(x, dtype=np.float32)
    W = np.ascontiguousarray(W, dtype=np.float32)

    order = np.argsort(dst, kind="stable")
    dst_s = dst[order]
    src_s = src[order]

    core_lo = np.searchsorted(dst_s, np.arange(N_CORES) * NODES_PER_CORE)
    core_hi = np.searchsorted(dst_s, (np.arange(N_CORES) + 1) * NODES_PER_CORE)

    max_edges = int((core_hi - core_lo).max())
    n_groups = max(1, -(-max_edges // GROUP))  # ceil; slack via greedy fallback
    e_cap = n_groups * GROUP
    n_segs = n_groups * GROUP_SEGS
    n_chunks = e_cap // CHUNK

    x_bf = x.astype(BF16)
    eb_bf = np.asarray(edge_basis).astype(BF16)
    WT_h = np.ascontiguousarray(W.T.astype(BF16))  # [128, 64]
    iota_h = np.tile(
        np.arange(WIN, dtype=np.float32).astype(BF16), (128, 2 * SEG_CHUNKS, 1)
    )

    in_maps = []
    sides = []
    for c in range(N_CORES):
        lo, hi = core_lo[c], core_hi[c]
        n_real = hi - lo
        ldst_c = dst_s[lo:hi] - c * NODES_PER_CORE
        src_c = src_s[lo:hi]
        eb_idx = order[lo:hi]

        bases, rel, slot_of_edge = _segment_bases(ldst_c, n_segs, e_cap)

        # slot -> edge id (or -1)
        slot_edge = np.full(e_cap, -1, dtype=np.int64)
        slot_edge[slot_of_edge] = np.arange(n_real)

        filled = slot_edge >= 0
        # ---- ebT: [128, e_cap] bf16, zero on padding ----
        eb_pad = np.zeros((e_cap, D_RADIAL), dtype=BF16)
        eb_pad[filled] = eb_bf[eb_idx[slot_edge[filled]]]
        ebT_c = np.ascontiguousarray(eb_pad.T)

        # ---- xgP: [128, n_chunks*64] bf16 (partition = edge-in-chunk) ----
        slot_src = np.zeros(e_cap, dtype=np.int64)
        slot_src[filled] = src_c[slot_edge[filled]]
        sp = slot_src.reshape(n_chunks, 128).T  # [128, n_chunks]
        xgP_c = np.ascontiguousarray(x_bf[sp].reshape(128, n_chunks * D_IN))

        # ---- ldstT: [128, n_chunks] bf16 ----
        ldstT_c = np.ascontiguousarray(rel.reshape(n_chunks, 128).T).astype(BF16)

        in_maps.append(
            {
                "ebT": ebT_c,
                "xgP": xgP_c,
                "WT": WT_h,
                "ldstT": ldstT_c,
                "iota": iota_h,
            }
        )

        # host-side bias term: hb[n] = sum_{e: dst=n} x[src_e] (f32 exact)
        xb = np.zeros((NODES_PER_CORE, D_IN), dtype=np.float32)
        if n_real > 0:
            runs = np.flatnonzero(np.diff(ldst_c)) + 1
            boundaries = np.concatenate(([0], runs))
            sums = np.add.reduceat(x[src_c], boundaries, axis=0)
            xb[ldst_c[boundaries]] = sums
        sides.append((bases, xb))
    return in_maps, sides, n_groups


def kernel(x, edge_basis, src, dst, W, b):
    from concourse.bass_utils import run_bass_kernel_spmd

    b = np.ascontiguousarray(b, dtype=np.float32)
    in_maps, sides, n_groups = _host_preprocess(x, edge_basis, src, dst, W)

    key = ("nc", n_groups)
    if key not in _CACHED:
        _CACHED[key] = _build_nc(n_groups)
    nc = _CACHED[key]
    _CACHED["nc"] = nc  # for profiling harnesses

    res = run_bass_kernel_spmd(nc, in_maps, core_ids=list(range(N_CORES)))

    n_segs = n_groups * GROUP_SEGS
    h = np.zeros((N_NODES, D_IN), dtype=np.float32)
    for c in range(N_CORES):
        slabs = np.asarray(res.results[c]["slabs"], dtype=np.float32)
        slabs = slabs.reshape(n_groups, WIN, GROUP_SEGS, D_IN)
        slabs = slabs.transpose(0, 2, 1, 3).reshape(n_segs, WIN, D_IN)
        bases, xb = sides[c]
        h_pad = np.zeros((NODES_PER_CORE + WIN, D_IN), dtype=np.float32)
        for s in range(n_segs):
            h_pad[bases[s]:bases[s] + WIN] += slabs[s]
        hc = h_pad[:NODES_PER_CORE]
        hc += xb * b
        h[c * NODES_PER_CORE:(c + 1) * NODES_PER_CORE] = hc
    return h
